# revision 23
# baseline (speedup 1.0000x reference)
"""BinaryBasicBlock Trainium2 kernel (8-core SPMD, data-parallel over batch).

Reference computation:
  out1 = relu(BN1(binconv(x, w1) * alpha1 * beta1))
  out  = relu(BN2(binconv(out1, w2) * alpha2 * beta2) + x)

where binconv centers the input per (n,c) over spatial dims, takes sign, and
convolves with sign(w) (3x3, stride 1, pad 1); BN uses batch statistics over
(N, H, W) (cross-core all-reduce).

Key simplification: batch norm is invariant to any global scalar scale s on
its input (mean and std both scale by s) except through the eps term:
(s*h - s*m)/sqrt(s^2*v + eps) = (h - m)/sqrt(v + eps/s^2).  With
s = alpha*beta ~ 0.03 and var(h) ~ 500 (half-count units), eps/s^2 perturbs
the result by ~4e-6 relative — far below the 2e-2 tolerance.  So alpha and
beta are never computed; BN runs directly on the half-counts with a tiny eps.

Implementation notes:
  - signs are +-1 (0 on knife-edge), conv done in fp8 e4m3 DoubleRow
    (K=256/matmul), fp32 PSUM accumulation: conv outputs exact integers.
  - conv is 9 shifted matmuls over a zero-padded [58x58] "slab"; each PSUM
    tile covers 8 output rows x 58 cols (464 <= 512, one bank).
  - counts stored as fp16 half-counts (exact; |count|/2 <= 1152).
  - BN stats via bn_stats/bn_aggr on half-counts; single [P,4] all-reduce
    per conv (sum, sumsq per channel-tile).
  - stage C fused: one DVE tensor_scalar (fp16) computes
    acc = sum(max(A1*h, -B1)) = sum(relu(A1*h+B1)) - HW*B1, so the sign
    bias is just -acc/HW; then one ACT Sign pass with scale=A1, i.e.
    sign(relu(A1*h+B1) - mean) without materializing relu (valid since
    mean(relu) >= 0 makes the negative branch sign -1 either way).
  - x tiles stay SBUF-resident from stage A through the final residual add
    (no reload); the final relu result is written over the dead x tile.
  - work spread over ACT/DVE/Pool so the PE matmul stream is critical path.
"""

import sys

sys.path.insert(0, "/opt/trn_rl_repo")

import numpy as np

import concourse.bass as bass
import concourse.bacc as bacc
import concourse.tile as tile
import concourse.mybir as mybir
from concourse import bass_isa
from concourse import bass_utils

# ---------------------------------------------------------------- constants
N_CORES = 8
NIMG = 4          # images per core (32 / 8)
C = 256
P = 128
CT = 2            # channel tiles (256 / 128)
H = W = 56
HW = H * W        # 3136
PADW = 58
SLAB = 3392       # padded-slab stride (>= 58*58+2, 16-aligned)
RG_ROWS = 8       # output rows per PSUM tile
NRG = 7           # row groups per image (56 / 8)
NFREE = RG_ROWS * PADW   # 464 (<= 512, one PSUM bank)
NVAL = RG_ROWS * W       # 448 valid outputs per PSUM tile
NTAP = 9
EPS_H = 1e-3      # eps in half-count units (see module docstring)
NCH = 32 * HW     # global per-channel count for BN stats
NLOC = NIMG * HW  # per-core per-channel count

F32 = mybir.dt.float32
F16 = mybir.dt.float16
FP8 = mybir.dt.float8e4
U32 = mybir.dt.uint32

FP8_NP = mybir.dt.np(FP8)

AX = mybir.AxisListType
ALU = mybir.AluOpType
ACTF = mybir.ActivationFunctionType


def _rhs_off(rg: int, dy: int, dx: int) -> int:
    # output rows y0..y0+7; rhs element j maps to padded input
    # [(y0+1+dy)*58 + 1 + dx] + j
    return (rg * RG_ROWS + 1 + dy) * PADW + 1 + dx


# evac engine per (m*NRG+rg) index: ACT x9, DVE x5 (GpSimd can't read PSUM)
_EVAC_PLAN = ["a", "d", "a", "a", "d", "a", "a",
              "d", "a", "a", "d", "a", "d", "a"]


def _conv_img(nc, psum, wall, slab, cnt_n, bnst, n, cv_tag):
    """One image of one binary conv: 9-tap DoubleRow matmuls, evacuation
    rotated over ACT/DVE, bn_stats (DVE) right behind each evac."""
    w5 = wall.rearrange("p (m t j c) -> p m t j c", m=CT, t=NTAP, j=CT)
    slab3 = slab.rearrange("p (j s) -> p j s", j=CT)
    for m in range(CT):
        ptiles = []
        for rg in range(NRG):
            ptile = psum.tile([P, NFREE], F32,
                              name=f"pt_{cv_tag}_{n}_{m}_{rg}", tag="pt")
            ptiles.append(ptile)
        for tap in range(NTAP):
            dy, dx = tap // 3 - 1, tap % 3 - 1
            for rg in range(NRG):
                off = _rhs_off(rg, dy, dx)
                nc.tensor.matmul(
                    ptiles[rg][:, :],
                    lhsT=w5[:, m, tap],
                    rhs=slab3[:, :, off:off + NFREE],
                    start=(tap == 0),
                    stop=(tap == NTAP - 1),
                    perf_mode=mybir.MatmulPerfMode.DoubleRow,
                )
        for rg in range(NRG):
            pv = ptiles[rg].rearrange("p (r x) -> p r x", x=PADW)[:, :, 0:W]
            cslice = cnt_n[:, m * HW + rg * NVAL: m * HW + (rg + 1) * NVAL]
            cv = cslice.rearrange("p (r x) -> p r x", x=W)
            col = n * NRG + rg
            # evacuate as half-counts (exact in fp16)
            if _EVAC_PLAN[m * NRG + rg] == "a":
                nc.scalar.activation(cv, pv, ACTF.Copy, bias=0.0, scale=0.5)
            else:
                nc.vector.tensor_scalar_mul(cv, pv, 0.5)
            # per-channel partial stats of the half-counts
            nc.vector.bn_stats(
                bnst[:, (m * 28 + col) * 6: (m * 28 + col + 1) * 6],
                cslice,
            )


def _pack_ar(nc, bnst, aggr, mm_, ex_, arbuf):
    """bn_aggr per channel-tile, then pack [sum_t0,sum_t1,sumsq_t0,sumsq_t1]
    into arbuf[P,4] (sums over the local NLOC elements)."""
    for m in range(CT):
        nc.vector.bn_aggr(aggr[:, m, :], bnst[:, m * 168:(m + 1) * 168])
    # sum = N*mean ; sumsq = N*(var + mean^2)
    nc.vector.tensor_scalar(arbuf[:, 0:2], aggr[:, :, 0], float(NLOC), None,
                            op0=ALU.mult)
    nc.vector.tensor_tensor(mm_[:, :], aggr[:, :, 0], aggr[:, :, 0],
                            op=ALU.mult)
    nc.vector.tensor_tensor(ex_[:, :], aggr[:, :, 1], mm_[:, :], op=ALU.add)
    nc.vector.tensor_scalar(arbuf[:, 2:4], ex_[:, :], float(NLOC), None,
                            op0=ALU.mult)


def _bn_coeffs(nc, arres, gamma_sb, bnb_sb, cpool, tag):
    """From all-reduced [sum0, sum1, sumsq0, sumsq1] compute
    A = gamma * rsqrt(v + eps), B = bn_beta - A*mean per channel.
    rsqrt = ACT Sqrt of DVE reciprocal (Rsqrt activation is disallowed)."""
    m_h = cpool.tile([P, CT], F32, name=f"mh_{tag}", tag=f"mh_{tag}")
    nc.vector.tensor_scalar(m_h[:, :], arres[:, 0:2], 1.0 / NCH, None,
                            op0=ALU.mult)
    ex2 = cpool.tile([P, CT], F32, name=f"ex2_{tag}", tag=f"ex2_{tag}")
    nc.vector.tensor_scalar(ex2[:, :], arres[:, 2:4], 1.0 / NCH, None,
                            op0=ALU.mult)
    msq = cpool.tile([P, CT], F32, name=f"msq_{tag}", tag=f"msq_{tag}")
    nc.vector.tensor_tensor(msq[:, :], m_h[:, :], m_h[:, :], op=ALU.mult)
    v_h = cpool.tile([P, CT], F32, name=f"vh_{tag}", tag=f"vh_{tag}")
    nc.vector.tensor_tensor(v_h[:, :], ex2[:, :], msq[:, :], op=ALU.subtract)
    varg = cpool.tile([P, CT], F32, name=f"varg_{tag}", tag=f"varg_{tag}")
    nc.vector.tensor_scalar(varg[:, :], v_h[:, :], 1.0, EPS_H,
                            op0=ALU.mult, op1=ALU.add)
    rcp = cpool.tile([P, CT], F32, name=f"rcp_{tag}", tag=f"rcp_{tag}")
    nc.vector.reciprocal(rcp[:, :], varg[:, :])
    rsq = cpool.tile([P, CT], F32, name=f"rsq_{tag}", tag=f"rsq_{tag}")
    nc.scalar.activation(rsq[:, :], rcp[:, :], ACTF.Sqrt)
    A = cpool.tile([P, CT], F32, name=f"A_{tag}", tag=f"A_{tag}")
    nc.vector.tensor_tensor(A[:, :], rsq[:, :], gamma_sb[:, :], op=ALU.mult)
    amh = cpool.tile([P, CT], F32, name=f"amh_{tag}", tag=f"amh_{tag}")
    nc.vector.tensor_tensor(amh[:, :], A[:, :], m_h[:, :], op=ALU.mult)
    B = cpool.tile([P, CT], F32, name=f"B_{tag}", tag=f"B_{tag}")
    nc.vector.tensor_tensor(B[:, :], bnb_sb[:, :], amh[:, :], op=ALU.subtract)
    return A, B


def build_nc():
    nc = bacc.Bacc("TRN2", target_bir_lowering=False, debug=False,
                   num_devices=N_CORES)

    x_d = nc.dram_tensor("x", [NIMG, C, H, W], F32, kind="ExternalInput")
    WSZ = CT * NTAP * CT * P  # 4608
    w1_d = nc.dram_tensor("w1", [P, WSZ], FP8, kind="ExternalInput")
    w2_d = nc.dram_tensor("w2", [P, WSZ], FP8, kind="ExternalInput")
    g1_d = nc.dram_tensor("g1", [P, CT], F32, kind="ExternalInput")
    b1_d = nc.dram_tensor("b1", [P, CT], F32, kind="ExternalInput")
    g2_d = nc.dram_tensor("g2", [P, CT], F32, kind="ExternalInput")
    b2_d = nc.dram_tensor("b2", [P, CT], F32, kind="ExternalInput")
    out_d = nc.dram_tensor("out", [NIMG, C, H, W], F32, kind="ExternalOutput")

    with tile.TileContext(nc) as tc:
        with tc.tile_pool(name="persist", bufs=1) as persist, \
             tc.tile_pool(name="xio", bufs=4) as xio, \
             tc.tile_pool(name="zf", bufs=2) as zfp, \
             tc.tile_pool(name="small", bufs=12) as small, \
             tc.tile_pool(name="psum", bufs=8, space="PSUM") as psum, \
             tc.tile_pool(name="dram", bufs=1, space="DRAM") as dram:

            # ---- act-table preload: first ACT instruction triggers the
            # sqrt_and_others table-set load (covers Sign/Copy/Relu/Sqrt)
            dumm = persist.tile([P, 1], F32, tag="dumm")
            nc.vector.memset(dumm[:, :], 1.0)
            nc.scalar.activation(dumm[:, :], dumm[:, :], ACTF.Sqrt)

            # ---- w1 first on the sync ring (first matmul needs it), then
            # x tiles; image 0's tiles are CHUNKED across the three DMA
            # queues (sync/scalar/gpsimd) so they land in ~5us instead of
            # ~14us — the head is gated on reduce(sign(x0))
            w1sb = persist.tile([P, WSZ], FP8, tag="w1sb")
            nc.sync.dma_start(out=w1sb[:, :], in_=w1_d.ap())

            xa = {}
            for n in range(NIMG):
                for t in range(CT):
                    xa[(n, t)] = xio.tile([P, HW], F32, name=f"xa_{n}_{t}",
                                          tag="xio")
            # fp16 copies of x for the final residual add (enables the DVE
            # 2x mode in the tail; fp16 keeps |err| ~5e-4 relative)
            xb = {}
            for n in range(NIMG):
                for t in range(CT):
                    xb[(n, t)] = persist.tile([P, HW], F16,
                                              name=f"xb_{n}_{t}",
                                              tag=f"xb_{n}_{t}")
            slabs = [persist.tile([P, CT * SLAB], FP8, name=f"slab_{n}",
                                  tag=f"slab_{n}") for n in range(NIMG)]
            cnt = [persist.tile([P, CT * HW], F16, name=f"cnt_{n}",
                                tag=f"cnt_{n}") for n in range(NIMG)]
            # image 0 split into half-tiles on the two HWDGE rings (SWDGE
            # adds ~8-10us latency, so gpsimd only carries late images)
            for t in range(CT):
                for c, ring in ((0, nc.sync), (1, nc.scalar)):
                    r0, r1 = 28 * c, 28 * (c + 1)
                    ring.dma_start(
                        out=xa[(0, t)][:, r0 * W:r1 * W],
                        in_=x_d.ap()[0, t * P:(t + 1) * P, r0:r1])
            for n in range(NIMG):
                nc.gpsimd.memset(slabs[n][:, :].bitcast(U32), 0)
            nc.sync.dma_start(out=xa[(1, 0)][:, :], in_=x_d.ap()[1, 0:P])
            nc.scalar.dma_start(out=xa[(1, 1)][:, :], in_=x_d.ap()[1, P:2 * P])
            for n in (2, 3):
                nc.sync.dma_start(out=xa[(n, 0)][:, :],
                                  in_=x_d.ap()[n, 0:P])
                nc.gpsimd.dma_start(out=xa[(n, 1)][:, :],
                                    in_=x_d.ap()[n, P:2 * P])

            # early dummy all-reduce: absorbs any first-collective warmup /
            # core-skew rendezvous during conv1 instead of at AR1 (the CC
            # stream runs it concurrently; compute engines never wait on it)
            dram_d = dram.tile([P, 1], F32, tag="ard_in")
            dram_o = dram.tile([P, 1], F32, tag="ard_out")
            nc.sync.dma_start(out=dram_d[:, :], in_=dumm[:, :])
            nc.gpsimd.collective_compute(
                "AllReduce", ALU.add, replica_groups=[list(range(N_CORES))],
                ins=[dram_d.opt()], outs=[dram_o.opt()])

            # ---- stats buffers
            bnst1 = persist.tile([P, CT * 28 * 6], F32, tag="bnst1")
            bnst2 = persist.tile([P, CT * 28 * 6], F32, tag="bnst2")
            aggr1 = persist.tile([P, CT, 2], F32, tag="aggr1")
            aggr2 = persist.tile([P, CT, 2], F32, tag="aggr2")
            mm1 = persist.tile([P, CT], F32, tag="mm1")
            mm2 = persist.tile([P, CT], F32, tag="mm2")
            ex1 = persist.tile([P, CT], F32, tag="ex1")
            ex2b = persist.tile([P, CT], F32, tag="ex2b")
            arbuf1 = persist.tile([P, 4], F32, tag="arbuf1")
            arres1 = persist.tile([P, 4], F32, tag="arres1")
            arbuf2 = persist.tile([P, 4], F32, tag="arbuf2")
            arres2 = persist.tile([P, 4], F32, tag="arres2")
            ar1_in = dram.tile([P, 4], F32, tag="ar1_in")
            ar1_out = dram.tile([P, 4], F32, tag="ar1_out")
            ar2_in = dram.tile([P, 4], F32, tag="ar2_in")
            ar2_out = dram.tile([P, 4], F32, tag="ar2_out")
            acc = persist.tile([P, NIMG * CT], F32, tag="acc")
            bias_c = persist.tile([P, NIMG * CT], F32, tag="bias_c")

            # ---------------- stage A prep for one image -----------------
            # negm on ACT (Copy with scale) so the DVE queue holds only the
            # big reduces — a DMA-blocked reduce can't stall a tiny dep op
            def prep_a(n):
                for t in range(CT):
                    x_t = xa[(n, t)]
                    sums = small.tile([P, 1], F32, name=f"sA_{n}_{t}",
                                      tag="sm")
                    negm = small.tile([P, 1], F32, name=f"nA_{n}_{t}",
                                      tag="nm")
                    # free-axis reduce is DVE-only (GpSimd reduces C only)
                    nc.vector.tensor_reduce(sums[:, :], x_t[:, :], axis=AX.X,
                                            op=ALU.add)
                    nc.scalar.activation(negm[:, :], sums[:, :], ACTF.Copy,
                                         bias=0.0, scale=-1.0 / HW)
                    xv = x_t.rearrange("p (r x) -> p r x", x=W)
                    interior = slabs[n][:, t * SLAB + PADW + 1:
                                        t * SLAB + PADW + 1 + 56 * PADW]
                    sview = interior.rearrange("p (r x) -> p r x",
                                               x=PADW)[:, :, 0:W]
                    nc.scalar.activation(sview, xv, ACTF.Sign,
                                         bias=negm[:, :])
                # fp16 copy of image n-1's x for the tail — deferred one
                # image so it never delays the next image's sign in the
                # ACT queue
                if n > 0:
                    for t in range(CT):
                        nc.scalar.activation(xb[(n - 1, t)][:, :],
                                             xa[(n - 1, t)][:, :], ACTF.Copy)

            # ======= stage A + conv1, software-pipelined so image n+1's
            # prep sits ahead of image n's evacuations in each engine queue
            prep_a(0)
            prep_a(1)
            _conv_img(nc, psum, w1sb, slabs[0], cnt[0], bnst1, 0, "c1")
            # park non-urgent loads on the scalar ring (after act tables)
            w2sb = persist.tile([P, WSZ], FP8, tag="w2sb")
            nc.scalar.dma_start(out=w2sb[:, :], in_=w2_d.ap())
            g1sb = persist.tile([P, CT], F32, tag="g1sb")
            b1sb = persist.tile([P, CT], F32, tag="b1sb")
            g2sb = persist.tile([P, CT], F32, tag="g2sb")
            b2sb = persist.tile([P, CT], F32, tag="b2sb")
            for sb, dt_ in ((g1sb, g1_d), (b1sb, b1_d), (g2sb, g2_d),
                            (b2sb, b2_d)):
                nc.scalar.dma_start(out=sb[:, :], in_=dt_.ap())
            prep_a(2)
            _conv_img(nc, psum, w1sb, slabs[1], cnt[1], bnst1, 1, "c1")
            prep_a(3)
            _conv_img(nc, psum, w1sb, slabs[2], cnt[2], bnst1, 2, "c1")
            for t in range(CT):
                nc.scalar.activation(xb[(3, t)][:, :], xa[(3, t)][:, :],
                                     ACTF.Copy)
            _conv_img(nc, psum, w1sb, slabs[3], cnt[3], bnst1, 3, "c1")

            # ================= all-reduce 1 (BN1 stats)
            _pack_ar(nc, bnst1, aggr1, mm1, ex1, arbuf1)
            nc.sync.dma_start(out=ar1_in[:, :], in_=arbuf1[:, :])
            nc.gpsimd.collective_compute(
                "AllReduce", ALU.add, replica_groups=[list(range(N_CORES))],
                ins=[ar1_in.opt()], outs=[ar1_out.opt()])
            nc.sync.dma_start(out=arres1[:, :], in_=ar1_out[:, :])

            A1, B1 = _bn_coeffs(nc, arres1, g1sb, b1sb, persist, "bn1")

            # ---------------- stage C prep for one image -----------------
            # u = A1*h + B1 (DVE ts, fp16 4x); acc = sum(max(u,0)) (DVE ts
            # single-op with accumulator, 4x); sign bias = B1 - acc/HW via
            # ACT Identity; then sign(A1*h + bias) — equal to the reference
            # sign(relu(u) - mean(relu(u))) since mean(relu) >= 0
            def prep_c(n):
                for t in range(CT):
                    k = n * CT + t
                    h = cnt[n][:, t * HW:(t + 1) * HW]
                    u = zfp.tile([P, HW], F16, name=f"u_{n}_{t}", tag="z")
                    nc.vector.tensor_scalar(u[:, :], h, A1[:, t:t + 1],
                                            B1[:, t:t + 1],
                                            op0=ALU.mult, op1=ALU.add)
                    junk = zfp.tile([P, HW], F16, name=f"junk_{n}_{t}",
                                    tag="z")
                    # with accum_out, op1 is the accumulator's reduce op
                    nc.vector.tensor_scalar(junk[:, :], u[:, :], 0.0, None,
                                            op0=ALU.max, op1=ALU.add,
                                            accum_out=acc[:, k:k + 1])
                    nc.scalar.activation(bias_c[:, k:k + 1],
                                         acc[:, k:k + 1], ACTF.Identity,
                                         scale=-1.0 / HW,
                                         bias=B1[:, t:t + 1])
                    hv = h.rearrange("p (r x) -> p r x", x=W)
                    interior = slabs[n][:, t * SLAB + PADW + 1:
                                        t * SLAB + PADW + 1 + 56 * PADW]
                    sview = interior.rearrange("p (r x) -> p r x",
                                               x=PADW)[:, :, 0:W]
                    nc.scalar.activation(sview, hv, ACTF.Sign,
                                         scale=A1[:, t:t + 1],
                                         bias=bias_c[:, k:k + 1])

            # ======= stage C + conv2 (cnt reused for conv2 half-counts —
            # image n's cnt is dead once its signs are in the slab)
            prep_c(0)
            prep_c(1)
            _conv_img(nc, psum, w2sb, slabs[0], cnt[0], bnst2, 0, "c2")
            prep_c(2)
            _conv_img(nc, psum, w2sb, slabs[1], cnt[1], bnst2, 1, "c2")
            prep_c(3)
            _conv_img(nc, psum, w2sb, slabs[2], cnt[2], bnst2, 2, "c2")
            _conv_img(nc, psum, w2sb, slabs[3], cnt[3], bnst2, 3, "c2")

            # ================= all-reduce 2 (BN2 stats)
            _pack_ar(nc, bnst2, aggr2, mm2, ex2b, arbuf2)
            nc.sync.dma_start(out=ar2_in[:, :], in_=arbuf2[:, :])
            nc.gpsimd.collective_compute(
                "AllReduce", ALU.add, replica_groups=[list(range(N_CORES))],
                ins=[ar2_in.opt()], outs=[ar2_out.opt()])
            nc.sync.dma_start(out=arres2[:, :], in_=ar2_out[:, :])

            A2, B2 = _bn_coeffs(nc, arres2, g2sb, b2sb, persist, "bn2")

            # ================= final: out = relu(A2*h2 + B2 + x)
            # u = A2*h + B2 (DVE ts, fp16 4x); z = u + xb (DVE tt, fp16 2x);
            # relu halves on ACT feeding per-half DMA on alternating rings.
            # Output staging tiles recycle the long-dead xio pool slots.
            HH = HW // 2
            for n in range(NIMG):
                for t in range(CT):
                    k = n * CT + t
                    ob = xio.tile([P, HW], F32, name=f"ob_{n}_{t}", tag="xio")
                    u = zfp.tile([P, HW], F16, name=f"uf_{n}_{t}", tag="z")
                    nc.vector.tensor_scalar(
                        u[:, :], cnt[n][:, t * HW:(t + 1) * HW],
                        A2[:, t:t + 1], B2[:, t:t + 1],
                        op0=ALU.mult, op1=ALU.add)
                    z = zfp.tile([P, HW], F16, name=f"zf_{n}_{t}", tag="z")
                    nc.vector.tensor_tensor(z[:, :], u[:, :],
                                            xb[(n, t)][:, :], op=ALU.add)
                    for h in range(2):
                        sl = slice(h * HH, (h + 1) * HH)
                        nc.scalar.activation(ob[:, sl], z[:, sl], ACTF.Relu)
                        ring = nc.sync if (2 * k + h) % 2 == 0 else nc.gpsimd
                        ring.dma_start(
                            out=out_d.ap()[n, t * P:(t + 1) * P,
                                           h * 28:(h + 1) * 28],
                            in_=ob[:, sl])

    nc.compile()
    return nc


_NC_CACHE = None


def _get_nc():
    global _NC_CACHE
    if _NC_CACHE is None:
        _NC_CACHE = build_nc()
    return _NC_CACHE


def _pack_w(w: np.ndarray) -> np.ndarray:
    # [Cout, Cin, 3, 3] -> lhsT [128(k), CT(m), 9(tap), CT(j), 128(cout_inner)]
    ws = np.sign(w.astype(np.float32))
    ws = ws.reshape(CT, P, CT, P, NTAP // 3, 3)  # m, cout_in, j, k, ky, kx
    # -> k, m, (ky kx), j, cout_in
    ws = ws.transpose(3, 0, 4, 5, 2, 1).reshape(P, CT * NTAP * CT * P)
    return np.ascontiguousarray(ws).astype(FP8_NP)


def _pack_ch(v: np.ndarray) -> np.ndarray:
    # [256] -> [128, CT] (partition-major within each channel tile)
    return np.ascontiguousarray(np.asarray(v, np.float32).reshape(CT, P).T)


def kernel(x, conv1_w, alpha1, bn1_gamma, bn1_beta, conv2_w, alpha2,
           bn2_gamma, bn2_beta):
    nc = _get_nc()
    x = np.asarray(x, np.float32)
    w1p = _pack_w(np.asarray(conv1_w))
    w2p = _pack_w(np.asarray(conv2_w))
    g1 = _pack_ch(bn1_gamma)
    b1 = _pack_ch(bn1_beta)
    g2 = _pack_ch(bn2_gamma)
    b2 = _pack_ch(bn2_beta)

    in_maps = []
    for i in range(N_CORES):
        in_maps.append({
            "x": np.ascontiguousarray(x[i * NIMG:(i + 1) * NIMG]),
            "w1": w1p, "w2": w2p,
            "g1": g1, "b1": b1, "g2": g2, "b2": b2,
        })
    res = bass_utils.run_bass_kernel_spmd(nc, in_maps,
                                          core_ids=list(range(N_CORES)))
    out = np.concatenate([res.results[i]["out"] for i in range(N_CORES)],
                         axis=0)
    return out.astype(np.float32)


# revision 27
# speedup vs baseline: 1.1079x; 1.1079x over previous
"""BinaryBasicBlock Trainium2 kernel (8-core SPMD, data-parallel over batch).

Reference computation:
  out1 = relu(BN1(binconv(x, w1) * alpha1 * beta1))
  out  = relu(BN2(binconv(out1, w2) * alpha2 * beta2) + x)

where binconv centers the input per (n,c) over spatial dims, takes sign, and
convolves with sign(w) (3x3, stride 1, pad 1); BN uses batch statistics over
(N, H, W) (cross-core all-reduce).

Key simplification: batch norm is invariant to any global scalar scale s on
its input (mean and std both scale by s) except through the eps term:
(s*h - s*m)/sqrt(s^2*v + eps) = (h - m)/sqrt(v + eps/s^2).  With
s = alpha*beta ~ 0.03 and var(h) ~ 500 (half-count units), eps/s^2 perturbs
the result by ~4e-6 relative — far below the 2e-2 tolerance.  So alpha and
beta are never computed; BN runs directly on the half-counts with a tiny eps.

Implementation notes:
  - signs are +-1 (0 on knife-edge), conv done in fp8 e4m3 DoubleRow
    (K=256/matmul), fp32 PSUM accumulation: conv outputs exact integers.
  - conv is 9 shifted matmuls over a zero-padded [58x58] "slab"; each PSUM
    tile covers 8 output rows x 58 cols (464 <= 512, one bank).
  - counts stored as fp16 half-counts (exact; |count|/2 <= 1152).
  - BN stats via bn_stats/bn_aggr on half-counts; single [P,4] all-reduce
    per conv (sum, sumsq per channel-tile).
  - stage C fused: one DVE tensor_scalar (fp16) computes
    acc = sum(max(A1*h, -B1)) = sum(relu(A1*h+B1)) - HW*B1, so the sign
    bias is just -acc/HW; then one ACT Sign pass with scale=A1, i.e.
    sign(relu(A1*h+B1) - mean) without materializing relu (valid since
    mean(relu) >= 0 makes the negative branch sign -1 either way).
  - x tiles stay SBUF-resident from stage A through the final residual add
    (no reload); the final relu result is written over the dead x tile.
  - work spread over ACT/DVE/Pool so the PE matmul stream is critical path.
"""

import sys

sys.path.insert(0, "/opt/trn_rl_repo")

import numpy as np

import concourse.bass as bass
import concourse.bacc as bacc
import concourse.tile as tile
import concourse.mybir as mybir
from concourse import bass_isa
from concourse import bass_utils

# ---------------------------------------------------------------- constants
N_CORES = 8
NIMG = 4          # images per core (32 / 8)
C = 256
P = 128
CT = 2            # channel tiles (256 / 128)
H = W = 56
HW = H * W        # 3136
PADW = 58
SLAB = 3392       # padded-slab stride (>= 58*58+2, 16-aligned)
RG_ROWS = 8       # output rows per PSUM tile
NRG = 7           # row groups per image (56 / 8)
NFREE = RG_ROWS * PADW   # 464 (<= 512, one PSUM bank)
NVAL = RG_ROWS * W       # 448 valid outputs per PSUM tile
NTAP = 9
EPS_H = 1e-3      # eps in half-count units (see module docstring)
NCH = 32 * HW     # global per-channel count for BN stats
NLOC = NIMG * HW  # per-core per-channel count

F32 = mybir.dt.float32
F16 = mybir.dt.float16
FP8 = mybir.dt.float8e4
U32 = mybir.dt.uint32

FP8_NP = mybir.dt.np(FP8)

AX = mybir.AxisListType
ALU = mybir.AluOpType
ACTF = mybir.ActivationFunctionType


def _rhs_off(rg: int, dy: int, dx: int) -> int:
    # output rows y0..y0+7; rhs element j maps to padded input
    # [(y0+1+dy)*58 + 1 + dx] + j
    return (rg * RG_ROWS + 1 + dy) * PADW + 1 + dx


# evac engine per (m*NRG+rg) index: ACT x9, DVE x5 (GpSimd can't read PSUM)
_EVAC_PLAN = ["a", "d", "a", "a", "d", "a", "a",
              "d", "a", "a", "d", "a", "d", "a"]


def _conv_img(nc, psum, wall, slab, cnt_n, n, cv_tag):
    """One image of one binary conv: 9-tap DoubleRow matmuls, evacuation
    rotated over ACT/DVE.  bn_stats is emitted separately (one image
    later) so it never sits ahead of the next image's prep in the DVE
    queue — a bn_stats batch blocking there stalls the PE."""
    w5 = wall.rearrange("p (m t j c) -> p m t j c", m=CT, t=NTAP, j=CT)
    slab3 = slab.rearrange("p (j s) -> p j s", j=CT)
    for m in range(CT):
        ptiles = []
        for rg in range(NRG):
            ptile = psum.tile([P, NFREE], F32,
                              name=f"pt_{cv_tag}_{n}_{m}_{rg}", tag="pt")
            ptiles.append(ptile)
        for tap in range(NTAP):
            dy, dx = tap // 3 - 1, tap % 3 - 1
            for rg in range(NRG):
                off = _rhs_off(rg, dy, dx)
                nc.tensor.matmul(
                    ptiles[rg][:, :],
                    lhsT=w5[:, m, tap],
                    rhs=slab3[:, :, off:off + NFREE],
                    start=(tap == 0),
                    stop=(tap == NTAP - 1),
                    perf_mode=mybir.MatmulPerfMode.DoubleRow,
                )
        for rg in range(NRG):
            pv = ptiles[rg].rearrange("p (r x) -> p r x", x=PADW)[:, :, 0:W]
            cslice = cnt_n[:, m * HW + rg * NVAL: m * HW + (rg + 1) * NVAL]
            cv = cslice.rearrange("p (r x) -> p r x", x=W)
            # evacuate as half-counts (exact in fp16)
            if _EVAC_PLAN[m * NRG + rg] == "a":
                nc.scalar.activation(cv, pv, ACTF.Copy, bias=0.0, scale=0.5)
            else:
                nc.vector.tensor_scalar_mul(cv, pv, 0.5)


def _emit_bnst(nc, cnt_n, bnst, n):
    """Per-channel partial stats over image n's half-counts (DVE)."""
    for m in range(CT):
        for rg in range(NRG):
            col = n * NRG + rg
            nc.vector.bn_stats(
                bnst[:, (m * 28 + col) * 6: (m * 28 + col + 1) * 6],
                cnt_n[:, m * HW + rg * NVAL: m * HW + (rg + 1) * NVAL],
            )


def _pack_ar(nc, bnst, aggr, mm_, ex_, arbuf):
    """bn_aggr per channel-tile, then pack [sum_t0,sum_t1,sumsq_t0,sumsq_t1]
    into arbuf[P,4] (sums over the local NLOC elements)."""
    for m in range(CT):
        nc.vector.bn_aggr(aggr[:, m, :], bnst[:, m * 168:(m + 1) * 168])
    # sum = N*mean ; sumsq = N*(var + mean^2)
    nc.vector.tensor_scalar(arbuf[:, 0:2], aggr[:, :, 0], float(NLOC), None,
                            op0=ALU.mult)
    nc.vector.tensor_tensor(mm_[:, :], aggr[:, :, 0], aggr[:, :, 0],
                            op=ALU.mult)
    nc.vector.tensor_tensor(ex_[:, :], aggr[:, :, 1], mm_[:, :], op=ALU.add)
    nc.vector.tensor_scalar(arbuf[:, 2:4], ex_[:, :], float(NLOC), None,
                            op0=ALU.mult)


def _bn_coeffs(nc, arres, gamma_sb, bnb_sb, cpool, tag):
    """From all-reduced [sum0, sum1, sumsq0, sumsq1] compute
    A = gamma * rsqrt(v + eps), B = bn_beta - A*mean per channel.
    rsqrt = ACT Sqrt of DVE reciprocal (Rsqrt activation is disallowed)."""
    m_h = cpool.tile([P, CT], F32, name=f"mh_{tag}", tag=f"mh_{tag}")
    nc.vector.tensor_scalar(m_h[:, :], arres[:, 0:2], 1.0 / NCH, None,
                            op0=ALU.mult)
    ex2 = cpool.tile([P, CT], F32, name=f"ex2_{tag}", tag=f"ex2_{tag}")
    nc.vector.tensor_scalar(ex2[:, :], arres[:, 2:4], 1.0 / NCH, None,
                            op0=ALU.mult)
    msq = cpool.tile([P, CT], F32, name=f"msq_{tag}", tag=f"msq_{tag}")
    nc.vector.tensor_tensor(msq[:, :], m_h[:, :], m_h[:, :], op=ALU.mult)
    v_h = cpool.tile([P, CT], F32, name=f"vh_{tag}", tag=f"vh_{tag}")
    nc.vector.tensor_tensor(v_h[:, :], ex2[:, :], msq[:, :], op=ALU.subtract)
    varg = cpool.tile([P, CT], F32, name=f"varg_{tag}", tag=f"varg_{tag}")
    nc.vector.tensor_scalar(varg[:, :], v_h[:, :], 1.0, EPS_H,
                            op0=ALU.mult, op1=ALU.add)
    rcp = cpool.tile([P, CT], F32, name=f"rcp_{tag}", tag=f"rcp_{tag}")
    nc.vector.reciprocal(rcp[:, :], varg[:, :])
    rsq = cpool.tile([P, CT], F32, name=f"rsq_{tag}", tag=f"rsq_{tag}")
    nc.scalar.activation(rsq[:, :], rcp[:, :], ACTF.Sqrt)
    A = cpool.tile([P, CT], F32, name=f"A_{tag}", tag=f"A_{tag}")
    nc.vector.tensor_tensor(A[:, :], rsq[:, :], gamma_sb[:, :], op=ALU.mult)
    amh = cpool.tile([P, CT], F32, name=f"amh_{tag}", tag=f"amh_{tag}")
    nc.vector.tensor_tensor(amh[:, :], A[:, :], m_h[:, :], op=ALU.mult)
    B = cpool.tile([P, CT], F32, name=f"B_{tag}", tag=f"B_{tag}")
    nc.vector.tensor_tensor(B[:, :], bnb_sb[:, :], amh[:, :], op=ALU.subtract)
    return A, B


def build_nc():
    nc = bacc.Bacc("TRN2", target_bir_lowering=False, debug=False,
                   num_devices=N_CORES)

    x_d = nc.dram_tensor("x", [NIMG, C, H, W], F32, kind="ExternalInput")
    WSZ = CT * NTAP * CT * P  # 4608
    w1_d = nc.dram_tensor("w1", [P, WSZ], FP8, kind="ExternalInput")
    w2_d = nc.dram_tensor("w2", [P, WSZ], FP8, kind="ExternalInput")
    g1_d = nc.dram_tensor("g1", [P, CT], F32, kind="ExternalInput")
    b1_d = nc.dram_tensor("b1", [P, CT], F32, kind="ExternalInput")
    g2_d = nc.dram_tensor("g2", [P, CT], F32, kind="ExternalInput")
    b2_d = nc.dram_tensor("b2", [P, CT], F32, kind="ExternalInput")
    out_d = nc.dram_tensor("out", [NIMG, C, H, W], F32, kind="ExternalOutput")

    with tile.TileContext(nc) as tc:
        with tc.tile_pool(name="persist", bufs=1) as persist, \
             tc.tile_pool(name="xio", bufs=4) as xio, \
             tc.tile_pool(name="zf", bufs=2) as zfp, \
             tc.tile_pool(name="small", bufs=12) as small, \
             tc.tile_pool(name="psum", bufs=8, space="PSUM") as psum, \
             tc.tile_pool(name="dram", bufs=1, space="DRAM") as dram:

            # ---- act-table preload: first ACT instruction triggers the
            # sqrt_and_others table-set load (covers Sign/Copy/Relu/Sqrt)
            dumm = persist.tile([P, 1], F32, tag="dumm")
            nc.vector.memset(dumm[:, :], 1.0)
            nc.scalar.activation(dumm[:, :], dumm[:, :], ACTF.Sqrt)

            # ---- w1 first on the sync ring (first matmul needs it), then
            # x tiles; image 0's tiles are CHUNKED across the three DMA
            # queues (sync/scalar/gpsimd) so they land in ~5us instead of
            # ~14us — the head is gated on reduce(sign(x0))
            w1sb = persist.tile([P, WSZ], FP8, tag="w1sb")
            nc.sync.dma_start(out=w1sb[:, :], in_=w1_d.ap())

            xa = {}
            for n in range(NIMG):
                for t in range(CT):
                    xa[(n, t)] = xio.tile([P, HW], F32, name=f"xa_{n}_{t}",
                                          tag="xio")
            # fp16 copies of x for the final residual add (enables the DVE
            # 2x mode in the tail; fp16 keeps |err| ~5e-4 relative)
            xb = {}
            for n in range(NIMG):
                for t in range(CT):
                    xb[(n, t)] = persist.tile([P, HW], F16,
                                              name=f"xb_{n}_{t}",
                                              tag=f"xb_{n}_{t}")
            slabs = [persist.tile([P, CT * SLAB], FP8, name=f"slab_{n}",
                                  tag=f"slab_{n}") for n in range(NIMG)]
            cnt = [persist.tile([P, CT * HW], F16, name=f"cnt_{n}",
                                tag=f"cnt_{n}") for n in range(NIMG)]
            # image 0 split into half-tiles on the two HWDGE rings (SWDGE
            # adds ~8-10us latency, so gpsimd only carries late images)
            for t in range(CT):
                for c, ring in ((0, nc.sync), (1, nc.scalar)):
                    r0, r1 = 28 * c, 28 * (c + 1)
                    ring.dma_start(
                        out=xa[(0, t)][:, r0 * W:r1 * W],
                        in_=x_d.ap()[0, t * P:(t + 1) * P, r0:r1])
            for n in range(NIMG):
                nc.gpsimd.memset(slabs[n][:, :].bitcast(U32), 0)
            nc.sync.dma_start(out=xa[(1, 0)][:, :], in_=x_d.ap()[1, 0:P])
            nc.scalar.dma_start(out=xa[(1, 1)][:, :], in_=x_d.ap()[1, P:2 * P])
            for n in (2, 3):
                nc.sync.dma_start(out=xa[(n, 0)][:, :],
                                  in_=x_d.ap()[n, 0:P])
                nc.gpsimd.dma_start(out=xa[(n, 1)][:, :],
                                    in_=x_d.ap()[n, P:2 * P])

            # early dummy all-reduce: absorbs any first-collective warmup /
            # core-skew rendezvous during conv1 instead of at AR1 (the CC
            # stream runs it concurrently; compute engines never wait on it)
            dram_d = dram.tile([P, 1], F32, tag="ard_in")
            dram_o = dram.tile([P, 1], F32, tag="ard_out")
            nc.sync.dma_start(out=dram_d[:, :], in_=dumm[:, :])
            nc.gpsimd.collective_compute(
                "AllReduce", ALU.add, replica_groups=[list(range(N_CORES))],
                ins=[dram_d.opt()], outs=[dram_o.opt()])

            # ---- stats buffers
            bnst1 = persist.tile([P, CT * 28 * 6], F32, tag="bnst1")
            bnst2 = persist.tile([P, CT * 28 * 6], F32, tag="bnst2")
            aggr1 = persist.tile([P, CT, 2], F32, tag="aggr1")
            aggr2 = persist.tile([P, CT, 2], F32, tag="aggr2")
            mm1 = persist.tile([P, CT], F32, tag="mm1")
            mm2 = persist.tile([P, CT], F32, tag="mm2")
            ex1 = persist.tile([P, CT], F32, tag="ex1")
            ex2b = persist.tile([P, CT], F32, tag="ex2b")
            arbuf1 = persist.tile([P, 4], F32, tag="arbuf1")
            arres1 = persist.tile([P, 4], F32, tag="arres1")
            arbuf2 = persist.tile([P, 4], F32, tag="arbuf2")
            arres2 = persist.tile([P, 4], F32, tag="arres2")
            ar1_in = dram.tile([P, 4], F32, tag="ar1_in")
            ar1_out = dram.tile([P, 4], F32, tag="ar1_out")
            ar2_in = dram.tile([P, 4], F32, tag="ar2_in")
            ar2_out = dram.tile([P, 4], F32, tag="ar2_out")
            acc = persist.tile([P, NIMG * CT], F32, tag="acc")
            bias_c = persist.tile([P, NIMG * CT], F32, tag="bias_c")

            # ---------------- stage A prep for one image -----------------
            # negm on ACT (Copy with scale) so the DVE queue holds only the
            # big reduces — a DMA-blocked reduce can't stall a tiny dep op
            def prep_a(n):
                for t in range(CT):
                    x_t = xa[(n, t)]
                    sums = small.tile([P, 1], F32, name=f"sA_{n}_{t}",
                                      tag="sm")
                    negm = small.tile([P, 1], F32, name=f"nA_{n}_{t}",
                                      tag="nm")
                    # free-axis reduce is DVE-only (GpSimd reduces C only)
                    nc.vector.tensor_reduce(sums[:, :], x_t[:, :], axis=AX.X,
                                            op=ALU.add)
                    nc.scalar.activation(negm[:, :], sums[:, :], ACTF.Copy,
                                         bias=0.0, scale=-1.0 / HW)
                    xv = x_t.rearrange("p (r x) -> p r x", x=W)
                    interior = slabs[n][:, t * SLAB + PADW + 1:
                                        t * SLAB + PADW + 1 + 56 * PADW]
                    sview = interior.rearrange("p (r x) -> p r x",
                                               x=PADW)[:, :, 0:W]
                    nc.scalar.activation(sview, xv, ACTF.Sign,
                                         bias=negm[:, :])
                # fp16 copy of image n-1's x for the tail — on DVE (2x) and
                # deferred one image so it frees the xio slot without ever
                # delaying the next image's sign
                if n > 0:
                    for t in range(CT):
                        nc.vector.tensor_scalar_mul(xb[(n - 1, t)][:, :],
                                                    xa[(n - 1, t)][:, :], 1.0)

            # ======= stage A + conv1, software-pipelined so image n+1's
            # prep sits ahead of image n's evacuations in each engine queue
            prep_a(0)
            prep_a(1)
            _conv_img(nc, psum, w1sb, slabs[0], cnt[0], 0, "c1")
            # park non-urgent loads on the scalar ring (after act tables)
            w2sb = persist.tile([P, WSZ], FP8, tag="w2sb")
            nc.scalar.dma_start(out=w2sb[:, :], in_=w2_d.ap())
            g1sb = persist.tile([P, CT], F32, tag="g1sb")
            b1sb = persist.tile([P, CT], F32, tag="b1sb")
            g2sb = persist.tile([P, CT], F32, tag="g2sb")
            b2sb = persist.tile([P, CT], F32, tag="b2sb")
            for sb, dt_ in ((g1sb, g1_d), (b1sb, b1_d), (g2sb, g2_d),
                            (b2sb, b2_d)):
                nc.scalar.dma_start(out=sb[:, :], in_=dt_.ap())
            prep_a(2)
            _emit_bnst(nc, cnt[0], bnst1, 0)
            _conv_img(nc, psum, w1sb, slabs[1], cnt[1], 1, "c1")
            prep_a(3)
            _emit_bnst(nc, cnt[1], bnst1, 1)
            _conv_img(nc, psum, w1sb, slabs[2], cnt[2], 2, "c1")
            for t in range(CT):
                nc.vector.tensor_scalar_mul(xb[(3, t)][:, :],
                                            xa[(3, t)][:, :], 1.0)
            _emit_bnst(nc, cnt[2], bnst1, 2)
            _conv_img(nc, psum, w1sb, slabs[3], cnt[3], 3, "c1")
            _emit_bnst(nc, cnt[3], bnst1, 3)

            # ================= all-reduce 1 (BN1 stats)
            _pack_ar(nc, bnst1, aggr1, mm1, ex1, arbuf1)
            nc.sync.dma_start(out=ar1_in[:, :], in_=arbuf1[:, :])
            nc.gpsimd.collective_compute(
                "AllReduce", ALU.add, replica_groups=[list(range(N_CORES))],
                ins=[ar1_in.opt()], outs=[ar1_out.opt()])
            nc.sync.dma_start(out=arres1[:, :], in_=ar1_out[:, :])

            A1, B1 = _bn_coeffs(nc, arres1, g1sb, b1sb, persist, "bn1")

            # ---------------- stage C prep for one image -----------------
            # u = A1*h + B1 (DVE ts, fp16 4x); acc = sum(max(u,0)) (DVE ts
            # single-op with accumulator, 4x); sign bias = B1 - acc/HW via
            # ACT Identity; then sign(A1*h + bias) — equal to the reference
            # sign(relu(u) - mean(relu(u))) since mean(relu) >= 0
            def prep_c(n):
                for t in range(CT):
                    k = n * CT + t
                    h = cnt[n][:, t * HW:(t + 1) * HW]
                    u = zfp.tile([P, HW], F16, name=f"u_{n}_{t}", tag="z")
                    nc.vector.tensor_scalar(u[:, :], h, A1[:, t:t + 1],
                                            B1[:, t:t + 1],
                                            op0=ALU.mult, op1=ALU.add)
                    junk = zfp.tile([P, HW], F16, name=f"junk_{n}_{t}",
                                    tag="z")
                    # with accum_out, op1 is the accumulator's reduce op
                    nc.vector.tensor_scalar(junk[:, :], u[:, :], 0.0, None,
                                            op0=ALU.max, op1=ALU.add,
                                            accum_out=acc[:, k:k + 1])
                    nc.scalar.activation(bias_c[:, k:k + 1],
                                         acc[:, k:k + 1], ACTF.Identity,
                                         scale=-1.0 / HW,
                                         bias=B1[:, t:t + 1])
                    hv = h.rearrange("p (r x) -> p r x", x=W)
                    interior = slabs[n][:, t * SLAB + PADW + 1:
                                        t * SLAB + PADW + 1 + 56 * PADW]
                    sview = interior.rearrange("p (r x) -> p r x",
                                               x=PADW)[:, :, 0:W]
                    nc.scalar.activation(sview, hv, ACTF.Sign,
                                         scale=A1[:, t:t + 1],
                                         bias=bias_c[:, k:k + 1])

            # ======= stage C + conv2 (cnt reused for conv2 half-counts —
            # image n's cnt is dead once its signs are in the slab)
            prep_c(0)
            prep_c(1)
            _conv_img(nc, psum, w2sb, slabs[0], cnt[0], 0, "c2")
            prep_c(2)
            _emit_bnst(nc, cnt[0], bnst2, 0)
            _conv_img(nc, psum, w2sb, slabs[1], cnt[1], 1, "c2")
            prep_c(3)
            _emit_bnst(nc, cnt[1], bnst2, 1)
            _conv_img(nc, psum, w2sb, slabs[2], cnt[2], 2, "c2")
            _emit_bnst(nc, cnt[2], bnst2, 2)
            _conv_img(nc, psum, w2sb, slabs[3], cnt[3], 3, "c2")
            _emit_bnst(nc, cnt[3], bnst2, 3)

            # ================= all-reduce 2 (BN2 stats)
            _pack_ar(nc, bnst2, aggr2, mm2, ex2b, arbuf2)
            nc.sync.dma_start(out=ar2_in[:, :], in_=arbuf2[:, :])
            nc.gpsimd.collective_compute(
                "AllReduce", ALU.add, replica_groups=[list(range(N_CORES))],
                ins=[ar2_in.opt()], outs=[ar2_out.opt()])
            nc.sync.dma_start(out=arres2[:, :], in_=ar2_out[:, :])

            A2, B2 = _bn_coeffs(nc, arres2, g2sb, b2sb, persist, "bn2")

            # ================= final: out = relu(A2*h2 + B2 + x)
            # u = A2*h + B2 (DVE ts, fp16 4x); z = u + xb (DVE tt, fp16 2x);
            # relu halves on ACT feeding per-half DMA on alternating rings.
            # Output staging tiles recycle the long-dead xio pool slots.
            HH = HW // 2
            for n in range(NIMG):
                for t in range(CT):
                    k = n * CT + t
                    ob = xio.tile([P, HW], F32, name=f"ob_{n}_{t}", tag="xio")
                    u = zfp.tile([P, HW], F16, name=f"uf_{n}_{t}", tag="z")
                    nc.vector.tensor_scalar(
                        u[:, :], cnt[n][:, t * HW:(t + 1) * HW],
                        A2[:, t:t + 1], B2[:, t:t + 1],
                        op0=ALU.mult, op1=ALU.add)
                    z = zfp.tile([P, HW], F16, name=f"zf_{n}_{t}", tag="z")
                    nc.vector.tensor_tensor(z[:, :], u[:, :],
                                            xb[(n, t)][:, :], op=ALU.add)
                    for h in range(2):
                        sl = slice(h * HH, (h + 1) * HH)
                        nc.scalar.activation(ob[:, sl], z[:, sl], ACTF.Relu)
                        ring = nc.sync if (2 * k + h) % 2 == 0 else nc.gpsimd
                        ring.dma_start(
                            out=out_d.ap()[n, t * P:(t + 1) * P,
                                           h * 28:(h + 1) * 28],
                            in_=ob[:, sl])

    nc.compile()
    return nc


_NC_CACHE = None


def _get_nc():
    global _NC_CACHE
    if _NC_CACHE is None:
        _NC_CACHE = build_nc()
    return _NC_CACHE


def _pack_w(w: np.ndarray) -> np.ndarray:
    # [Cout, Cin, 3, 3] -> lhsT [128(k), CT(m), 9(tap), CT(j), 128(cout_inner)]
    ws = np.sign(w.astype(np.float32))
    ws = ws.reshape(CT, P, CT, P, NTAP // 3, 3)  # m, cout_in, j, k, ky, kx
    # -> k, m, (ky kx), j, cout_in
    ws = ws.transpose(3, 0, 4, 5, 2, 1).reshape(P, CT * NTAP * CT * P)
    return np.ascontiguousarray(ws).astype(FP8_NP)


def _pack_ch(v: np.ndarray) -> np.ndarray:
    # [256] -> [128, CT] (partition-major within each channel tile)
    return np.ascontiguousarray(np.asarray(v, np.float32).reshape(CT, P).T)


def kernel(x, conv1_w, alpha1, bn1_gamma, bn1_beta, conv2_w, alpha2,
           bn2_gamma, bn2_beta):
    nc = _get_nc()
    x = np.asarray(x, np.float32)
    w1p = _pack_w(np.asarray(conv1_w))
    w2p = _pack_w(np.asarray(conv2_w))
    g1 = _pack_ch(bn1_gamma)
    b1 = _pack_ch(bn1_beta)
    g2 = _pack_ch(bn2_gamma)
    b2 = _pack_ch(bn2_beta)

    in_maps = []
    for i in range(N_CORES):
        in_maps.append({
            "x": np.ascontiguousarray(x[i * NIMG:(i + 1) * NIMG]),
            "w1": w1p, "w2": w2p,
            "g1": g1, "b1": b1, "g2": g2, "b2": b2,
        })
    res = bass_utils.run_bass_kernel_spmd(nc, in_maps,
                                          core_ids=list(range(N_CORES)))
    out = np.concatenate([res.results[i]["out"] for i in range(N_CORES)],
                         axis=0)
    return out.astype(np.float32)
